# revision 11
# baseline (speedup 1.0000x reference)
"""Trainium2 Bass kernel for nn_Loss_40510131536268.

Algorithm
---------
The reference is a T-step normalized forward recursion over a fixed sparse
graph (E=16384 edges on V=2048 nodes), batched over B=32:

    log_C   = logsumexp(log_prev over out-nodes)
    prop    = exp(log_prev[:, out_idxs] - log_C)
    combined= scatter_add(prop -> in_idxs)
    log_curr= log_safe(combined) + x_t
    result  = log(sum over end nodes of exp(log_curr)) + sum(log_C)  at t+1==len

In probability space the per-step normalization by C cancels exactly in the
final result, so the recursion linearizes to

    U_t = (U_{t-1} @ A) * X_t        A[u,w] = #edges u->w,  X_t = exp(x_t)

with result[b] = log( sum_v U_{L-1}[b,v] * end_w[v] ) (+ exact bookkeeping for
any scales folded into X to keep fp32/bf16 in range).  The EPS clamps of the
reference only affect mass at relative level e^-64 — invisible in fp32.

Device work per step: 64 accumulating matmuls (lhsT = state tiles [128,4]
bf16, rhs = A tiles [128,512] bf16, fp32 PSUM), 4 DVE multiplies by the
streamed X tile, 16 PE transposes to rebuild the lhsT layout, one DMA of the
16KB state to HBM (E_t and the final select are cheap host post-processing).
Sharding: data-parallel over B (4 batch rows per core, indices/graph
replicated), 8 cores, no collectives.
"""

import os
import numpy as np
import ml_dtypes

bf16 = ml_dtypes.bfloat16

V, B, T, E, S = 2048, 32, 256, 16384, 128
NCORES = 8
BL = B // NCORES        # 4 batch rows per core
NK = V // 128           # 16 contraction tiles
NJ = V // 512           # 4 psum banks of N=512
EPS = float(np.exp(-64.0))

_PROGRAM_CACHE = {}


def _split_multi_waits(nc):
    """walrus in this toolchain rejects compute instructions carrying more
    than one semaphore wait ("Too many sync wait commands").  Split extra
    waits onto no-op instructions inserted immediately before, on the same
    engine (engine-local program order preserves the gating semantics)."""
    import concourse.mybir as mybir

    skip = (
        mybir.InstCall,
        mybir.InstUnconditionalBranch,
        mybir.InstCompareAndBranch,
        mybir.InstIndirectBranch,
        mybir.InstHalt,
    )
    for f in nc.m.functions:
        for blk in f.blocks:
            out = []
            changed = False
            for inst in blk.instructions:
                si = inst.sync_info
                if (
                    si is not None
                    and si.on_wait
                    and len(si.on_wait) > 1
                    and not isinstance(inst, skip)
                ):
                    waits = list(si.on_wait)
                    for w in waits[:-1]:
                        out.append(
                            mybir.InstNoOp(
                                name=nc.get_next_instruction_name(),
                                engine=inst.engine,
                                ins=[],
                                outs=[],
                                bass_nofuse=True,
                                sync_info=mybir.SyncInfo(on_wait=[w], on_update=[]),
                            )
                        )
                    inst.sync_info = mybir.SyncInfo(
                        on_wait=[waits[-1]], on_update=list(si.on_update or [])
                    )
                    changed = True
                out.append(inst)
            if changed:
                blk.instructions = out


def build_program(n_steps, split_waits=True, calib_steps=None):
    """Build the SPMD Bass/Tile program (identical on all 8 cores).

    calib_steps: if set, execute only that many steps while keeping all
    DRAM tensor shapes identical (timing-calibration variant)."""
    import concourse.bass as bass
    import concourse.mybir as mybir
    from concourse.tile import TileContext

    f32 = mybir.dt.float32
    b16 = mybir.dt.bfloat16

    nc = bass.Bass()
    a_in = nc.declare_dram_parameter("a_rhs", [128, NK * V], b16, isOutput=False)
    u0_in = nc.declare_dram_parameter("u0t", [128, NK * BL], b16, isOutput=False)
    xs_in = nc.declare_dram_parameter("xs", [n_steps * BL, V], b16, isOutput=False)
    id_in = nc.declare_dram_parameter("ident", [BL, BL], b16, isOutput=False)
    uh_out = nc.declare_dram_parameter(
        "u_hist", [n_steps * 128, NK * BL], b16, isOutput=True
    )

    with TileContext(nc) as tc:
        with (
            tc.tile_pool(name="const", bufs=1) as cpool,
            tc.tile_pool(name="xv", bufs=3) as xpool,
            tc.tile_pool(name="pr", bufs=4, space="PSUM") as prpool,
            tc.tile_pool(name="pt", bufs=2, space="PSUM") as ptpool,
        ):
            a_sb = cpool.tile([128, NK * V], b16, tag="a_sb")
            nc.gpsimd.dma_start(a_sb[:, :], a_in[:, :])
            u_sb = cpool.tile([128, NK * BL], b16, tag="u_sb")
            nc.gpsimd.dma_start(u_sb[:, :], u0_in[:, :])
            ident = cpool.tile([BL, BL], b16, tag="ident")
            nc.gpsimd.dma_start(ident[:, :], id_in[:, :])
            un_flat = cpool.tile([BL, V], b16, tag="un_flat")

            for i in range(calib_steps if calib_steps is not None else n_steps):
                # stream in X'_t (bf16 [4, 2048])
                x_sb = xpool.tile([BL, V], b16, tag="x_sb")
                nc.gpsimd.dma_start(x_sb[:, :], xs_in[i * BL : (i + 1) * BL, :])

                pt = ptpool.tile([128, NK * BL], b16, tag="pt")
                for j in range(NJ):
                    r_j = prpool.tile([BL, 512], f32, tag="r")
                    for k in range(NK):
                        nc.tensor.matmul(
                            r_j[:, :],
                            u_sb[:, k * BL : (k + 1) * BL],
                            a_sb[:, k * V + j * 512 : k * V + (j + 1) * 512],
                            start=(k == 0),
                            stop=(k == NK - 1),
                        )
                    # U_next chunk = R * X  (bf16 out)
                    nc.vector.tensor_mul(
                        un_flat[:, j * 512 : (j + 1) * 512],
                        r_j[:, :],
                        x_sb[:, j * 512 : (j + 1) * 512],
                    )
                    # rebuild transposed state tiles for this chunk
                    for k in range(4 * j, 4 * (j + 1)):
                        nc.tensor.transpose(
                            pt[:, k * BL : (k + 1) * BL],
                            un_flat[:, k * 128 : (k + 1) * 128],
                            ident[:, :],
                        )
                nc.vector.tensor_copy(u_sb[:, :], pt[:, :])
                nc.gpsimd.dma_start(
                    uh_out[i * 128 : (i + 1) * 128, :], u_sb[:, :]
                )
    if split_waits:
        _split_multi_waits(nc)
    return nc


def _prep_host(inputs, n_steps):
    """Host-side preprocessing shared by all cores."""
    x = np.asarray(inputs["extracted_log_probs"], np.float32)   # [V,B,T]
    in_idxs = np.asarray(inputs["in_idxs"]).astype(np.int64)
    out_idxs = np.asarray(inputs["out_idxs"]).astype(np.int64)
    start_idxs = np.asarray(inputs["start_idxs"]).astype(np.int64)
    end_idxs = np.asarray(inputs["end_idxs"]).astype(np.int64)

    xt = np.ascontiguousarray(np.transpose(x, (2, 1, 0)))       # [T,B,V]

    A_cnt = np.zeros((V, V), np.float32)
    np.add.at(A_cnt, (out_idxs, in_idxs), 1.0)

    end_w = np.zeros((V,), np.float32)
    np.add.at(end_w, end_idxs, 1.0)

    start_mask = np.zeros((V,), bool)
    start_mask[start_idxs] = True

    # A tiles for the rhs: a_sb[p, k*V + w] = A_cnt[k*128+p, w]
    a_sb = np.ascontiguousarray(
        A_cnt.reshape(NK, 128, V).transpose(1, 0, 2).reshape(128, NK * V)
    ).astype(bf16)

    # U_0 = exp(log_curr0)
    X0 = np.exp(xt[0])                                           # [B,V]
    U0 = np.where(start_mask[None, :], X0, np.float32(EPS)).astype(np.float32)
    U0_16 = U0.astype(bf16)

    # scales sigma_t[b] folded into X' (t = 1..n_steps)
    Xall = np.exp(xt[1 : n_steps + 1])                           # [n,B,V]
    m = Xall.mean(axis=2)                                        # [n,B]
    sigma = (1.0 / (8.0 * m)).astype(np.float32)
    cumlog = np.cumsum(np.log(sigma.astype(np.float64)), axis=0) # [n,B]
    Xs16 = (Xall * sigma[:, :, None]).astype(bf16)               # [n,B,V]

    return dict(a_sb=a_sb, U0_16=U0_16, Xs16=Xs16, cumlog=cumlog, end_w=end_w)


def _core_inputs(prep, core, n_steps):
    bsl = slice(core * BL, (core + 1) * BL)
    # u0t[p, k*BL+b] = U0[b, k*128+p]
    u0c = prep["U0_16"][bsl]                                     # [BL, V]
    u0t = np.ascontiguousarray(
        u0c.reshape(BL, NK, 128).transpose(2, 1, 0).reshape(128, NK * BL)
    )
    xs = np.ascontiguousarray(prep["Xs16"][:, bsl, :].reshape(n_steps * BL, V))
    return {
        "a_rhs": prep["a_sb"],
        "u0t": u0t,
        "xs": xs,
        "ident": np.eye(BL, dtype=bf16),
    }


def _postprocess(prep, results, target_lengths, n_steps):
    """results: list of per-core out_maps with 'u_hist'."""
    end_w_kp = prep["end_w"].reshape(NK, 128)                    # [k, p]
    E_dev = np.zeros((n_steps + 1, B), np.float64)
    # t = 0 from host U0 (bf16-rounded, same as device state precision)
    E_dev[0] = prep["U0_16"].astype(np.float32) @ prep["end_w"]
    for c in range(NCORES):
        uh = np.asarray(results[c]["u_hist"]).reshape(n_steps, 128, NK, BL)
        # E[t, b] = sum_{k,p} uh[t, p, k, b] * end_w[k*128+p]
        Ec = np.einsum("tpkb,kp->tb", uh.astype(np.float32), end_w_kp)
        E_dev[1:, c * BL : (c + 1) * BL] = Ec

    lengths = np.asarray(target_lengths).astype(np.int64)
    res = np.zeros((B,), np.float64)
    for b in range(B):
        L = int(lengths[b])
        corr = prep["cumlog"][L - 2, b] if L >= 2 else 0.0
        res[b] = np.log(E_dev[L - 1, b]) - corr
    return (-res).astype(np.float32)


def run_on_device(nc, core_maps, **kwargs):
    from concourse.bass_utils import run_bass_kernel_spmd

    return run_bass_kernel_spmd(nc, core_maps, core_ids=list(range(NCORES)), **kwargs)


def kernel(**inputs) -> np.ndarray:
    n_steps = T - 1
    prep = _prep_host(inputs, n_steps)
    key = n_steps
    if key not in _PROGRAM_CACHE:
        _PROGRAM_CACHE[key] = build_program(n_steps)
    nc = _PROGRAM_CACHE[key]
    core_maps = [_core_inputs(prep, c, n_steps) for c in range(NCORES)]
    out = run_on_device(nc, core_maps)
    return _postprocess(prep, out.results, inputs["target_lengths"], n_steps)


# revision 15
# speedup vs baseline: 2734.4820x; 2734.4820x over previous
"""Trainium2 Bass kernel for nn_Loss_40510131536268.

Algorithm
---------
The reference is a T-step normalized forward recursion over a fixed sparse
graph (E=16384 edges on V=2048 nodes), batched over B=32:

    log_C   = logsumexp(log_prev over out-nodes)
    prop    = exp(log_prev[:, out_idxs] - log_C)
    combined= scatter_add(prop -> in_idxs)
    log_curr= log_safe(combined) + x_t
    result  = log(sum over end nodes of exp(log_curr)) + sum(log_C)  at t+1==len

In probability space the per-step normalization by C cancels exactly in the
final result, so the recursion linearizes to

    U_t = (U_{t-1} @ A) * X_t        A[u,w] = #edges u->w,  X_t = exp(x_t)

with result[b] = log( sum_v U_{L-1}[b,v] * end_w[v] ) (+ exact bookkeeping for
any scales folded into X to keep fp32/bf16 in range).  The EPS clamps of the
reference only affect mass at relative level e^-64 — invisible in fp32.

Device work per step: 64 accumulating matmuls (lhsT = state tiles [128,4]
bf16, rhs = A tiles [128,512] bf16, fp32 PSUM), 4 DVE multiplies by the
streamed X tile, 16 PE transposes to rebuild the lhsT layout, one DMA of the
16KB state to HBM (E_t and the final select are cheap host post-processing).
Sharding: data-parallel over B (4 batch rows per core, indices/graph
replicated), 8 cores, no collectives.
"""

import os
import numpy as np
import ml_dtypes

bf16 = ml_dtypes.bfloat16

V, B, T, E, S = 2048, 32, 256, 16384, 128
NCORES = 8
BL = B // NCORES        # 4 batch rows per core
NK = V // 128           # 16 contraction tiles
NJ = V // 512           # 4 psum banks of N=512
EPS = float(np.exp(-64.0))

_PROGRAM_CACHE = {}


def _split_multi_waits(nc):
    """walrus in this toolchain rejects compute instructions carrying more
    than one semaphore wait ("Too many sync wait commands").  Split extra
    waits onto no-op instructions inserted immediately before, on the same
    engine (engine-local program order preserves the gating semantics)."""
    import concourse.mybir as mybir

    skip = (
        mybir.InstCall,
        mybir.InstUnconditionalBranch,
        mybir.InstCompareAndBranch,
        mybir.InstIndirectBranch,
        mybir.InstHalt,
    )
    for f in nc.m.functions:
        for blk in f.blocks:
            out = []
            changed = False
            for inst in blk.instructions:
                si = inst.sync_info
                if (
                    si is not None
                    and si.on_wait
                    and len(si.on_wait) > 1
                    and not isinstance(inst, skip)
                ):
                    waits = list(si.on_wait)
                    for w in waits[:-1]:
                        out.append(
                            mybir.InstNoOp(
                                name=nc.get_next_instruction_name(),
                                engine=inst.engine,
                                ins=[],
                                outs=[],
                                bass_nofuse=True,
                                sync_info=mybir.SyncInfo(on_wait=[w], on_update=[]),
                            )
                        )
                    inst.sync_info = mybir.SyncInfo(
                        on_wait=[waits[-1]], on_update=list(si.on_update or [])
                    )
                    changed = True
                out.append(inst)
            if changed:
                blk.instructions = out


def build_program(n_steps, split_waits=True, calib_steps=None):
    """Build the SPMD Bass/Tile program (identical on all 8 cores).

    calib_steps: if set, execute only that many steps while keeping all
    DRAM tensor shapes identical (timing-calibration variant)."""
    import concourse.bass as bass
    import concourse.mybir as mybir
    from concourse.tile import TileContext

    f32 = mybir.dt.float32
    b16 = mybir.dt.bfloat16

    nc = bass.Bass()
    a_in = nc.declare_dram_parameter("a_rhs", [128, NK * V], b16, isOutput=False)
    u0_in = nc.declare_dram_parameter("u0t", [128, NK * BL], b16, isOutput=False)
    xs_in = nc.declare_dram_parameter("xs", [n_steps * BL, V], b16, isOutput=False)
    id_in = nc.declare_dram_parameter("ident", [BL, BL], b16, isOutput=False)
    uh_out = nc.declare_dram_parameter(
        "u_hist", [n_steps * 128, NK * BL], b16, isOutput=True
    )

    with TileContext(nc) as tc:
        with (
            tc.tile_pool(name="const", bufs=1) as cpool,
            tc.tile_pool(name="xv", bufs=3) as xpool,
            tc.tile_pool(name="pr", bufs=4, space="PSUM") as prpool,
            tc.tile_pool(name="pt", bufs=2, space="PSUM") as ptpool,
        ):
            a_sb = cpool.tile([128, NK * V], b16, tag="a_sb")
            nc.gpsimd.dma_start(a_sb[:, :], a_in[:, :])
            ident = cpool.tile([BL, BL], b16, tag="ident")
            nc.gpsimd.dma_start(ident[:, :], id_in[:, :])
            # state quarters: u_q[q] holds k-tiles 4q..4q+3 (v in [512q, 512q+512)).
            # Separate tiles so cross-step deps are per-quarter: step t+1's
            # early matmuls only need quarter 0, which lands long before
            # quarter 3 — the PE never drains between steps.
            QW = 4 * BL
            u_q = [
                [
                    cpool.tile([128, QW], b16, tag=f"u_q{p}{q}", name=f"u_q{p}{q}")
                    for q in range(NJ)
                ]
                for p in range(2)
            ]
            for q in range(NJ):
                nc.gpsimd.dma_start(u_q[0][q][:, :], u0_in[:, q * QW : (q + 1) * QW])
            un_q = [cpool.tile([BL, 512], b16, tag=f"un_q{q}", name=f"un_q{q}") for q in range(NJ)]

            for i in range(calib_steps if calib_steps is not None else n_steps):
                # stream in X'_t (bf16 [4, 2048])
                x_sb = xpool.tile([BL, V], b16, tag="x_sb")
                nc.sync.dma_start(x_sb[:, :], xs_in[i * BL : (i + 1) * BL, :])

                rd = u_q[i % 2]
                wr = u_q[(i + 1) % 2]
                for j in range(NJ):
                    r_j = prpool.tile([BL, 512], f32, tag="r")
                    for k in range(NK):
                        nc.tensor.matmul(
                            r_j[:, :],
                            rd[k // 4][:, (k % 4) * BL : (k % 4 + 1) * BL],
                            a_sb[:, k * V + j * 512 : k * V + (j + 1) * 512],
                            start=(k == 0),
                            stop=(k == NK - 1),
                        )
                    # U_next chunk = R * X  (bf16 out)
                    nc.vector.tensor_mul(
                        un_q[j][:, :],
                        r_j[:, :],
                        x_sb[:, j * 512 : (j + 1) * 512],
                    )
                    # rebuild this quarter's transposed state tiles
                    pt_j = ptpool.tile([128, QW], b16, tag="pt")
                    for kk in range(4):
                        nc.tensor.transpose(
                            pt_j[:, kk * BL : (kk + 1) * BL],
                            un_q[j][:, kk * 128 : (kk + 1) * 128],
                            ident[:, :],
                        )
                    nc.vector.tensor_copy(wr[j][:, :], pt_j[:, :])
                    nc.sync.dma_start(
                        uh_out[i * 128 : (i + 1) * 128, j * QW : (j + 1) * QW],
                        wr[j][:, :],
                    )
    if split_waits:
        _split_multi_waits(nc)
    return nc


def _prep_host(inputs, n_steps):
    """Host-side preprocessing shared by all cores."""
    x = np.asarray(inputs["extracted_log_probs"], np.float32)   # [V,B,T]
    in_idxs = np.asarray(inputs["in_idxs"]).astype(np.int64)
    out_idxs = np.asarray(inputs["out_idxs"]).astype(np.int64)
    start_idxs = np.asarray(inputs["start_idxs"]).astype(np.int64)
    end_idxs = np.asarray(inputs["end_idxs"]).astype(np.int64)

    xt = np.ascontiguousarray(np.transpose(x, (2, 1, 0)))       # [T,B,V]

    A_cnt = np.zeros((V, V), np.float32)
    np.add.at(A_cnt, (out_idxs, in_idxs), 1.0)

    end_w = np.zeros((V,), np.float32)
    np.add.at(end_w, end_idxs, 1.0)

    start_mask = np.zeros((V,), bool)
    start_mask[start_idxs] = True

    # A tiles for the rhs: a_sb[p, k*V + w] = A_cnt[k*128+p, w]
    a_sb = np.ascontiguousarray(
        A_cnt.reshape(NK, 128, V).transpose(1, 0, 2).reshape(128, NK * V)
    ).astype(bf16)

    # U_0 = exp(log_curr0)
    X0 = np.exp(xt[0])                                           # [B,V]
    U0 = np.where(start_mask[None, :], X0, np.float32(EPS)).astype(np.float32)
    U0_16 = U0.astype(bf16)

    # scales sigma_t[b] folded into X' (t = 1..n_steps)
    Xall = np.exp(xt[1 : n_steps + 1])                           # [n,B,V]
    m = Xall.mean(axis=2)                                        # [n,B]
    sigma = (1.0 / (8.0 * m)).astype(np.float32)
    cumlog = np.cumsum(np.log(sigma.astype(np.float64)), axis=0) # [n,B]
    Xs16 = (Xall * sigma[:, :, None]).astype(bf16)               # [n,B,V]

    return dict(a_sb=a_sb, U0_16=U0_16, Xs16=Xs16, cumlog=cumlog, end_w=end_w)


def _core_inputs(prep, core, n_steps):
    bsl = slice(core * BL, (core + 1) * BL)
    # u0t[p, k*BL+b] = U0[b, k*128+p]
    u0c = prep["U0_16"][bsl]                                     # [BL, V]
    u0t = np.ascontiguousarray(
        u0c.reshape(BL, NK, 128).transpose(2, 1, 0).reshape(128, NK * BL)
    )
    xs = np.ascontiguousarray(prep["Xs16"][:, bsl, :].reshape(n_steps * BL, V))
    return {
        "a_rhs": prep["a_sb"],
        "u0t": u0t,
        "xs": xs,
        "ident": np.eye(BL, dtype=bf16),
    }


def _postprocess(prep, results, target_lengths, n_steps):
    """results: list of per-core out_maps with 'u_hist'."""
    end_w_kp = prep["end_w"].reshape(NK, 128)                    # [k, p]
    E_dev = np.zeros((n_steps + 1, B), np.float64)
    # t = 0 from host U0 (bf16-rounded, same as device state precision)
    E_dev[0] = prep["U0_16"].astype(np.float32) @ prep["end_w"]
    for c in range(NCORES):
        uh = np.asarray(results[c]["u_hist"]).reshape(n_steps, 128, NK, BL)
        # E[t, b] = sum_{k,p} uh[t, p, k, b] * end_w[k*128+p]
        Ec = np.einsum("tpkb,kp->tb", uh.astype(np.float32), end_w_kp)
        E_dev[1:, c * BL : (c + 1) * BL] = Ec

    lengths = np.asarray(target_lengths).astype(np.int64)
    res = np.zeros((B,), np.float64)
    for b in range(B):
        L = int(lengths[b])
        corr = prep["cumlog"][L - 2, b] if L >= 2 else 0.0
        res[b] = np.log(E_dev[L - 1, b]) - corr
    return (-res).astype(np.float32)


def run_on_device(nc, core_maps, **kwargs):
    from concourse.bass_utils import run_bass_kernel_spmd

    return run_bass_kernel_spmd(nc, core_maps, core_ids=list(range(NCORES)), **kwargs)


def kernel(**inputs) -> np.ndarray:
    n_steps = T - 1
    prep = _prep_host(inputs, n_steps)
    key = n_steps
    if key not in _PROGRAM_CACHE:
        _PROGRAM_CACHE[key] = build_program(n_steps)
    nc = _PROGRAM_CACHE[key]
    core_maps = [_core_inputs(prep, c, n_steps) for c in range(NCORES)]
    out = run_on_device(nc, core_maps)
    return _postprocess(prep, out.results, inputs["target_lengths"], n_steps)
